# revision 31
# baseline (speedup 1.0000x reference)
"""Trainium2 Bass kernel for nn_DEFNet: 16-branch 1D conv (k=3..33) + bias + ReLU
+ channel-mean over x[32, 1, 262144] -> out[32, 262144].   ~95.5us on 8 cores.

Strategy (per core, 8 cores, 4 batch rows each):
  - Host builds a transposed sliding-window view xwinT[k, t] = xpad[64t + k]
    (k in [0,96)) plus a constant-1.0 row k=96, so each channel-pair's conv
    WITH BIAS is ONE matmul:
       psum[(c,p), t] = sum_k lhsT[k, 64c+p] * xwinT[k, t]
                      = (conv_{2j+c}(x)[64t+p] + b_{2j+c}) / 16
    (mean and bias folded into the weights; row 96 of lhsT = b/16, so every
    drain op is a pure max-with-0 / relu, no per-pair bias operands).
  - Per 1024-segment granule, 8 pair-psums ([128,1024] = 2 banks, pool of 4
    -> all 8 banks, drained immediately so the PE can run ahead):
      * ScalarE relus the 4 even pairs -> rt tiles (activation, no bias AP)
      * VectorE drains the 4 odd pairs with scalar_tensor_tensor
        c_k = max(ps,0) + rt_k  (consumes the matching ScalarE tile; ops are
        independent, no serial chain, so engines stay decoupled)
      * GpSimd (which CANNOT read PSUM) merges c-tiles pairwise (TT add)
        into 2 partial planes; the host sums the planes + folds halves.
    This puts ~71us of irreducible PSUM-drain on each of V/A (the two
    PSUM-capable engines at ~1.04-1.14 ns/col), ~68us of merge on Pool, and
    keeps the PE mostly at full 2.4GHz p-state (~213ns/512-col matmul).

Hardware findings this design is built on (TRN2, measured):
  - fp16/bf16 matmul: 512-col takes 427ns at the 1.2GHz mid p-state, 213ns
    at 2.4GHz; the PE only ramps up when continuously busy (~3us), so PSUM
    slot recycling latency directly sets PE speed. LDWEIGHTS overlaps.
  - fp8e4(+DoubleRow, 2x PE) fails this problem's 2e-2 gate: measured
    3.8e-2 end-to-end (numpy sim on the real inputs); hybrids don't help
    because PE time is output-column-bound, not MAC-bound.
  - DVE 2x/4x perf modes from the cost model do NOT materialize on HW for
    TT/STT (bf16 SBUF TT measured ~1.39ns/col, same as 1x).
  - GpSimd cannot access PSUM (BIR verifier) and only lowers
    TensorTensor/TensorCopy (TensorScalarPtr rejected); TT add runs at 0.42
    efficiency (~2.1us per [128,1024]).
  - DMA: a DRAM->SBUF start with >96 partitions pins ALL its descriptors to
    one DMA engine (~25GB/s); <=96-partition chunks spread across all 16.
    DMA is descriptor-rate bound (~140-160ns per 2KB descriptor), so each
    extra output plane costs ~18us of per-engine DMA busy -> 2 planes max.
    SBUF->SBUF accumulate-DMA (cce_op=add) works but RMW triples bus volume
    and the traffic slows V/A ~20% via SBUF port contention.
"""

import os

import numpy as np

import concourse.bass as bass
import concourse.mybir as mybir
import concourse.tile as tile
from concourse import bacc, bass_utils
from concourse.tile import TileContext

B, L = 32, 262144
NCONV, MAXK = 16, 33
NCORES = 8
ROWS = B // NCORES          # batch rows per core
P = 64                      # output positions per segment
W = 97                      # window rows (96) + constant-1 bias row
HALO = 16
T = L // P                  # segments per row (4096)

BLK = 1024                  # segments per granule (psum pair-tile free dim)
MMN = 512                   # matmul free size (one 2KB psum bank)

DT_X = mybir.dt.float16
DT_W = mybir.dt.float16
DT_E = mybir.dt.bfloat16    # relu/accumulate dtype
F32 = mybir.dt.float32


def _support_mask():
    m = np.zeros((NCONV, MAXK), dtype=np.float32)
    c = MAXK // 2
    for i in range(1, NCONV + 1):
        m[i - 1, c - i:c + i + 1] = 1.0
    return m


def _build_lhsT(w, b):
    """[97, 8*128] f32; pair j cols j*128..(j+1)*128,
    lhsT[k, 64c+p] = wm[2j+c, k-p]/16 for k<96; lhsT[96, 64c+p] = b[2j+c]/16."""
    wm = (np.asarray(w, np.float32) * _support_mask()) / 16.0
    bs = np.asarray(b, np.float32) / 16.0
    lhsT = np.zeros((W, 8 * 128), dtype=np.float32)
    for j in range(8):
        for c in range(2):
            ch = 2 * j + c
            for p in range(P):
                lhsT[p:p + MAXK, j * 128 + c * 64 + p] = wm[ch]
            lhsT[96, j * 128 + c * 64:j * 128 + c * 64 + P] = bs[ch]
    return lhsT


def _build_nc():
    nc = bacc.Bacc(
        "TRN2",
        target_bir_lowering=False,
        debug=False,
        enable_asserts=False,
        num_devices=NCORES,
    )
    xwin = nc.dram_tensor("xwin", [ROWS * W, T], DT_X, kind="ExternalInput").ap()
    wts = nc.dram_tensor("wts", [W, 8 * 128], DT_W, kind="ExternalInput").ap()
    # 2 partial planes (DMA is descriptor-rate bound: each extra plane
    # costs ~18us of per-engine DMA time); the final sum happens on host
    outH = nc.dram_tensor(
        "outH", [2 * ROWS * 128, T], DT_E, kind="ExternalOutput").ap()

    n_g = T // BLK              # granules per row (4)
    op_max, op_add = mybir.AluOpType.max, mybir.AluOpType.add
    relu = mybir.ActivationFunctionType.Relu

    with TileContext(nc) as tc:
        with (
            tc.tile_pool(name="consts", bufs=1) as cpool,
            tc.tile_pool(name="xin", bufs=ROWS) as xpool,
            tc.tile_pool(name="psum", bufs=4, space="PSUM") as pspool,
            tc.tile_pool(name="relu", bufs=8) as rpool,
            tc.tile_pool(name="vdrain", bufs=8) as vpool,
            tc.tile_pool(name="merge", bufs=6) as mpool,
        ):
            # NOTE: DMAs with >96 partitions pin all descriptors to one DMA
            # engine; split every 97-row transfer into [96,...] + [1,...].
            # All issued on the SP sequencer, row-0-critical ones first.
            # issue order = ready order on the serialized SP sequencer:
            # row-0 chunk first, then weights, then the rest.
            w_sb = cpool.tile([W, 8 * 128], DT_W)
            x_sb = []
            for r in range(ROWS):
                xt = xpool.tile([W, T], DT_X, tag="xt")
                x_sb.append(xt)
            nc.sync.dma_start(
                x_sb[0][:96, 0:2048], xwin[0:96, 0:2048])
            nc.sync.dma_start(w_sb[:96, :], wts[:96, :])
            nc.sync.dma_start(
                x_sb[0][96:97, :], xwin[96:97, :])
            nc.sync.dma_start(w_sb[96:97, :], wts[96:97, :])
            nc.sync.dma_start(
                x_sb[0][:96, 2048:T], xwin[0:96, 2048:T])
            for r in range(1, ROWS):
                nc.sync.dma_start(
                    x_sb[r][:96, 0:2048], xwin[r * W:r * W + 96, 0:2048])
                nc.sync.dma_start(
                    x_sb[r][96:97, :], xwin[r * W + 96:(r + 1) * W, :])
                nc.sync.dma_start(
                    x_sb[r][:96, 2048:T], xwin[r * W:r * W + 96, 2048:T])

            for r in range(ROWS):
                # the very last granule runs as two 512-col halves so the
                # end-of-kernel serial chain (mm -> relu -> STT -> merge ->
                # DMA) is half as deep; its merges run on V (P is slower).
                work = [(g * BLK, BLK) for g in range(n_g)]
                if r == ROWS - 1:
                    work = work[:-1] + [(T - BLK, 512), (T - 512, 512)]
                for s0, blk in work:
                    final = (r == ROWS - 1 and s0 + blk == T)
                    # GpSimd cannot read PSUM: ScalarE relus even pairs to
                    # rt tiles; VectorE drains odd pairs with STT that adds
                    # the matching rt (c_k = max(ps,0) + rt_k, a natural
                    # one-op lag, no serial chain). Pool merges the four c
                    # tiles pairwise into 2 planes; host sums the planes.
                    rts, cs = [], []
                    for j in range(8):
                        ps = pspool.tile([128, BLK], F32)
                        lhsT = w_sb[:, j * 128:(j + 1) * 128]
                        for m in range(blk // MMN):
                            rhs = x_sb[r][:, s0 + m * MMN:s0 + (m + 1) * MMN]
                            nc.tensor.matmul(
                                ps[:, m * MMN:(m + 1) * MMN], lhsT, rhs,
                                start=True, stop=True)
                        if j % 2 == 0:
                            rt = rpool.tile([128, BLK], DT_E)
                            nc.scalar.activation(
                                rt[:, :blk], ps[:, :blk], relu)
                            rts.append(rt)
                        else:
                            c = vpool.tile([128, BLK], DT_E)
                            nc.vector.scalar_tensor_tensor(
                                c[:, :blk], ps[:, :blk], 0.0,
                                rts[j // 2][:, :blk], op_max, op_add)
                            cs.append(c)
                    for h in range(2):
                        m = mpool.tile([128, BLK], DT_E)
                        if final:
                            nc.vector.scalar_tensor_tensor(
                                m[:, :blk], cs[2 * h][:, :blk], 0.0,
                                cs[2 * h + 1][:, :blk], op_add, op_add)
                        else:
                            nc.gpsimd.tensor_tensor(
                                m[:, :blk], cs[2 * h][:, :blk],
                                cs[2 * h + 1][:, :blk], op_add)
                        r0 = (h * ROWS + r) * 128
                        nc.sync.dma_start(
                            outH[r0:r0 + 128, s0:s0 + blk], m[:, :blk])
    nc.compile()
    return nc


_NC_CACHE = None


def _get_nc():
    global _NC_CACHE
    if _NC_CACHE is None:
        _NC_CACHE = _build_nc()
    return _NC_CACHE


LAST_RESULTS = None


def _install_ntff_hook():
    """Provide antenv.axon_hooks (absent on this image) so
    run_bass_kernel_spmd(trace=True) can capture NTFF profiles via the
    axon PJRT plugin's C ABI. Also stub the artifact upload (no bucket
    creds in-container)."""
    import contextlib
    import ctypes
    import sys
    import types

    try:
        from antenv.axon_hooks import get_axon_ntff_profile_hook  # noqa: F401
        return  # real module present
    except ImportError:
        pass

    so_path = "/opt/axon/libaxon_pjrt.so"
    lib = ctypes.CDLL(so_path)
    lib.axon_start_nrt_profile.argtypes = [
        ctypes.POINTER(ctypes.c_int64), ctypes.c_size_t]
    lib.axon_start_nrt_profile.restype = ctypes.c_int64
    lib.axon_stop_nrt_profile.argtypes = [ctypes.c_char_p]
    lib.axon_stop_nrt_profile.restype = ctypes.c_int64

    @contextlib.contextmanager
    def _hook(output_dir, device_ids):
        import jax
        jax.devices()
        if device_ids:
            ids = (ctypes.c_int64 * len(device_ids))(*device_ids)
            rc = lib.axon_start_nrt_profile(ids, len(device_ids))
        else:
            rc = lib.axon_start_nrt_profile(None, 0)
        if rc != 0:
            raise RuntimeError(f"axon_start_nrt_profile rc={rc}")
        try:
            yield
        finally:
            n = lib.axon_stop_nrt_profile(str(output_dir).encode())
            print(f"ntff profile: {n} file(s) -> {output_dir}")

    mod = types.ModuleType("antenv.axon_hooks")
    mod.get_axon_ntff_profile_hook = lambda: _hook
    mod.set_axon_ntff_profile_hook = lambda h: None
    sys.modules["antenv.axon_hooks"] = mod
    bass_utils.upload_artifacts = lambda tmpdir: f"file://{tmpdir}"


def host_inputs(x, w, b):
    """Build the 8 per-core input maps from the full problem inputs."""
    x = np.asarray(x, np.float32)
    xpad = np.pad(x[:, 0, :], ((0, 0), (HALO, HALO)))  # [B, L+32]
    s = xpad.strides
    np_x = mybir.dt.np(DT_X)
    xwinT = np.lib.stride_tricks.as_strided(
        xpad, shape=(B, 96, T), strides=(s[0], s[1], P * s[1]))

    lhsT = _build_lhsT(w, b).astype(mybir.dt.np(DT_W))

    in_maps = []
    for core in range(NCORES):
        xw = np.empty((ROWS, W, T), dtype=np_x)
        xw[:, :96, :] = xwinT[core * ROWS:(core + 1) * ROWS]
        xw[:, 96, :] = 1.0
        in_maps.append({
            "xwin": xw.reshape(ROWS * W, T),
            "wts": lhsT,
        })
    return in_maps


def kernel(x, w, b):
    global LAST_RESULTS
    in_maps = host_inputs(x, w, b)
    nc = _get_nc()
    trace = bool(os.environ.get("KERNEL_TRACE"))
    if trace:
        _install_ntff_hook()
    res = bass_utils.run_bass_kernel_spmd(
        nc, in_maps, core_ids=list(range(NCORES)), trace=trace,
        **({"trace_cores": [0]} if trace else {}),
    )
    LAST_RESULTS = res

    out = np.empty((B, L), dtype=np.float32)
    for core in range(NCORES):
        oH = res.results[core]["outH"].reshape(2, ROWS, 2, P, T)
        folded = oH.astype(np.float32).sum(axis=(0, 2))       # [ROWS, P, T]
        for r in range(ROWS):
            out[core * ROWS + r] = folded[r].T.reshape(L)
    return out
